# revision 6
# baseline (speedup 1.0000x reference)
"""Trainium2 Bass kernel for nn_FCVI_Net_78864189489850.

Computation (reference):
  L = lower-tri scatter of cov_vector (exp on diag)          [769, 769]
  samples = mean + L @ z                                      [769, S, B]
  W0 = samples[0:256], b0 = samples[256:512],
  W1 = samples[512:768], b1 = samples[768]
  h = relu(x * W0 + b0);  out = sum_o h * W1 + b1             [S, B]

Strategy (8 NeuronCores, batch-sharded, no cross-device comms):
  - Host builds L (cheap scatter + 769 exps), transposes to LT, casts to
    f16.  Each core gets a B-shard of z (columns c = s*256 + b_local,
    4096 cols) in f16, PLUS 256 extra rows x*z[0:256] ("z0") so the PE
    accumulates x*sT0 + sT1 straight into one PSUM region.
  - Transposed-orientation matmuls: sT[c, i] = sum_k z[k,c] * LT[k,i]
    with exact triangular column trimming (2688 streamed PE columns per
    128-column c-tile).  Per c-tile, 12 matmuls:
      pA[128,256] = x*sT0 + sT1:  z0k0 (N=256), z0k1 (N=128),
                    zk0/zk1/zk2 (N=256 each, LT cols 256:512), zk3 (N=128)
      pB[128,256] = sT2:          zk0..zk4 (N=256, LT cols 512:768),
                    zk5 (N=128)
  - Epilogue (GpSimd cannot touch PSUM; DVE reads one PSUM operand/op):
      DVE:    u2 = pA + apar;  gA junk-mult with accum stagA = sum h*sT2
      GpSimd: h = relu(u2);    gBt = h * mean2
      ACT:    stagB[:,m] = sum_i gBt  (Copy with accum_out)
  - The b1 row (mean768 + L[768,:] @ z) is a host-side bias added in
    _assemble; it is 0.13% of the FLOPs.
  - Output staged [128, 32], DMA'd out in 4 chunks; host reassembles
    [16, 2048] and adds b1.
"""
import os
import numpy as np

P = 769
S = 16
B = 2048
NCORES = 8
BC = B // NCORES          # 256 batch per core
NCOL = S * BC             # 4096 columns per core
NCT = NCOL // 128         # 32 c-tiles per core
ZR = 1024                 # za rows: 768 z + 256 x-scaled z

# LT columns kept per k-tile t (LT[k, i] == 0 for i < k; col 768 on host)
LT_COLS = [(0, 768), (128, 768), (256, 768), (384, 768),
           (512, 768), (640, 768)]
# z column chunks: small head so compute starts early, 1024-wide steady
CHUNKS = [(0, 128), (128, 128), (256, 256), (512, 512),
          (1024, 1024), (2048, 1024), (3072, 1024)]

_cache = {}


def _mm_dtype():
    import concourse.mybir as mybir
    name = os.environ.get("BASS_FCVI_DTYPE", "f16")
    return {
        "f16": (mybir.dt.float16, np.float16),
        "f32r": (mybir.dt.float32r, np.float32),
    }[name]


def _build_program():
    import concourse.bacc as bacc
    import concourse.tile as tile
    from concourse import mybir

    mmdt, _ = _mm_dtype()
    f32 = mybir.dt.float32

    nc = bacc.Bacc("TRN2", target_bir_lowering=False, debug=False)

    za_d = nc.dram_tensor("za", [ZR, NCOL], mmdt, kind="ExternalInput")
    lt_d = nc.dram_tensor("lt", [768, P], mmdt, kind="ExternalInput")
    cst_d = nc.dram_tensor("cst", [128, 768], f32, kind="ExternalInput")
    out_d = nc.dram_tensor("out", [128, NCT], f32, kind="ExternalOutput")

    with tile.TileContext(nc) as tc:
        with (
            tc.tile_pool(name="zpool", bufs=1) as zpool,
            tc.tile_pool(name="ltpool", bufs=1) as ltpool,
            tc.tile_pool(name="cpool", bufs=1) as cpool,
            tc.tile_pool(name="work", bufs=4) as work,
            tc.tile_pool(name="gsc", bufs=3) as gsc,
            tc.tile_pool(name="pa", bufs=3, space="PSUM") as pa_pool,
            tc.tile_pool(name="pb", bufs=4, space="PSUM") as pb_pool,
        ):
            ltt = [None] * 6
            zc = [None] * len(CHUNKS)

            def load_lt(t, eng):
                lo, hi = LT_COLS[t]
                tl = ltpool.tile([128, hi - lo], mmdt, tag=f"lt{t}")
                eng.dma_start(
                    out=tl[:], in_=lt_d.ap()[t * 128:(t + 1) * 128, lo:hi])
                ltt[t] = tl

            def load_zc(q, eng):
                cs, cn = CHUNKS[q]
                zq = zpool.tile([128, 8, cn], mmdt, tag=f"zc{q}")
                src = za_d.ap()[:, cs:cs + cn].rearrange(
                    "(t p) c -> p t c", p=128)
                eng.dma_start(out=zq[:], in_=src)
                zc[q] = zq

            # Early DMAs issued from idle engines in parallel so c-tile 0's
            # dependencies (lt0, z cols 0..128, then remaining lt) land fast.
            load_lt(0, nc.scalar)
            load_zc(0, nc.sync)
            load_lt(1, nc.gpsimd)
            load_lt(2, nc.scalar)
            load_lt(3, nc.gpsimd)
            load_lt(4, nc.scalar)
            load_lt(5, nc.gpsimd)

            cst = cpool.tile([128, 768], f32, tag="cst")
            nc.scalar.dma_start(out=cst[:], in_=cst_d.ap()[:, :])
            apar = [cst[:, 0:256], cst[:, 256:512]]
            m2b = cst[:, 512:768]

            load_zc(1, nc.sync)
            load_zc(2, nc.sync)
            for q in range(3, len(CHUNKS)):
                load_zc(q, nc.sync)

            stagA = cpool.tile([128, NCT], f32, tag="stagA")
            stagB = cpool.tile([128, NCT], f32, tag="stagB")
            stag2 = cpool.tile([128, NCT], f32, tag="stag2")

            def rhs(t, g0, g1):
                lo, _ = LT_COLS[t]
                return ltt[t][:, g0 - lo:g1 - lo]

            qi = 0
            for m in range(NCT):
                par = m % 2
                if m * 128 >= CHUNKS[qi][0] + CHUNKS[qi][1]:
                    qi += 1
                cl = m * 128 - CHUNKS[qi][0]

                def lhsT(t):
                    return zc[qi][:, t, cl:cl + 128]

                pA = pa_pool.tile([128, 256], f32, tag="pA")
                pB = pb_pool.tile([128, 256], f32, tag="pB")
                # pA = x*sT0 + sT1 (exact triangular trim per k-tile):
                #   x*sT0 via host-prescaled z0 rows (slots 6,7),
                #   sT1 via z k-tiles 0..3 against LT cols [256, 512)
                nc.tensor.matmul(pA[:, 0:256], lhsT(6), rhs(0, 0, 256),
                                 start=True, stop=False)
                nc.tensor.matmul(pA[:, 128:256], lhsT(7),
                                 ltt[1][:, 0:128], start=False, stop=False)
                nc.tensor.matmul(pA[:, 0:256], lhsT(0), rhs(0, 256, 512),
                                 start=False, stop=False)
                nc.tensor.matmul(pA[:, 0:256], lhsT(1), rhs(1, 256, 512),
                                 start=False, stop=False)
                nc.tensor.matmul(pA[:, 0:256], lhsT(2), rhs(2, 256, 512),
                                 start=False, stop=False)
                nc.tensor.matmul(pA[:, 128:256], lhsT(3), rhs(3, 384, 512),
                                 start=False, stop=True)
                # pB = sT2 (samples rows [512, 768)): k-tiles 0..5, k5 trimmed
                for t in range(5):
                    nc.tensor.matmul(pB[:, 0:256], lhsT(t), rhs(t, 512, 768),
                                     start=(t == 0), stop=False)
                nc.tensor.matmul(pB[:, 128:256], lhsT(5), rhs(5, 640, 768),
                                 start=False, stop=True)

                # u2 = pA + (x*mean0 + mean1)   (DVE, host-precomputed apar)
                u2 = work.tile([128, 256], f32, tag="u2")
                nc.vector.tensor_add(u2[:], pA[:, 0:256], apar[par])
                # h = relu(u2)   (GpSimd)
                h = work.tile([128, 256], f32, tag="h")
                nc.gpsimd.tensor_scalar_max(h[:], u2[:], 0.0)
                # gA = sT2 * h ; stagA[:, m] = sum_o gA   (DVE, PSUM operand)
                gA = gsc.tile([128, 256], f32, tag="gA")
                nc.vector.scalar_tensor_tensor(
                    out=gA[:], in0=pB[:, 0:256], scalar=1.0, in1=h[:],
                    op0=mybir.AluOpType.mult, op1=mybir.AluOpType.mult,
                    accum_out=stagA[:, m:m + 1])
                # gBt = h * mean2   (GpSimd) ; stagB[:, m] = sum_o gBt  (ACT)
                gBt = gsc.tile([128, 256], f32, tag="gBt")
                nc.gpsimd.tensor_tensor(out=gBt[:], in0=h[:], in1=m2b,
                                        op=mybir.AluOpType.mult)
                jB = gsc.tile([128, 256], f32, tag="jB")
                nc.scalar.activation(jB[:], gBt[:],
                                     mybir.ActivationFunctionType.Copy,
                                     accum_out=stagB[:, m:m + 1])

                if m % 8 == 7:
                    sl = slice(m - 7, m + 1)
                    nc.vector.tensor_add(stag2[:, sl], stagA[:, sl],
                                         stagB[:, sl])
                    nc.sync.dma_start(out=out_d.ap()[:, sl], in_=stag2[:, sl])

    nc.compile()
    return nc


def _prep_inputs(x, mean, cov_vector, z):
    _, npdt = _mm_dtype()

    L = np.zeros((P, P), dtype=np.float32)
    L[np.tril_indices(P)] = cov_vector
    d = np.diag(L).copy()
    L[np.diag_indices(P)] = np.exp(d)

    lt = np.ascontiguousarray(L.T[:768]).astype(npdt)     # rows k in [0, 768)

    z2 = z.reshape(P, S, B).astype(np.float32, copy=False)
    # b1 bias row on host: b1[s, b] = mean[768] + sum_k L[768, k] z[k, s, b]
    b1 = mean[768] + np.tensordot(L[768, :], z2, axes=1)  # [S, B]

    in_maps = []
    for c in range(NCORES):
        zs = z2[:, :, c * BC:(c + 1) * BC].reshape(P, NCOL)
        xs = x[c * BC:(c + 1) * BC].astype(np.float32)
        za = np.empty((ZR, NCOL), dtype=npdt)
        za[:768] = zs[:768].astype(npdt)
        # x-prescaled z rows: column c = s*BC + b pairs with x[b]
        xcol = np.tile(xs, S)                             # [NCOL]
        za[768:1024] = (zs[:256] * xcol[None, :]).astype(npdt)
        cst = np.empty((128, 768), dtype=np.float32)
        # apar[p, i] = x_p * mean0_i + mean1_i, one block per batch parity
        cst[:, 0:256] = xs[0:128, None] * mean[None, 0:256] \
            + mean[None, 256:512]
        cst[:, 256:512] = xs[128:256, None] * mean[None, 0:256] \
            + mean[None, 256:512]
        cst[:, 512:768] = mean[None, 512:768]
        in_maps.append({"za": za, "lt": lt, "cst": cst})
    return in_maps, b1


def _assemble(results, b1):
    out = np.empty((S, B), dtype=np.float32)
    for c in range(NCORES):
        o = results[c]["out"]                       # [128, 32]
        oc = o.reshape(128, S, 2).transpose(1, 2, 0).reshape(S, BC)
        out[:, c * BC:(c + 1) * BC] = oc
    out += b1
    return out


def _run(inputs, trace=False, trace_kwargs=None):
    from concourse.bass_utils import run_bass_kernel_spmd

    key = os.environ.get("BASS_FCVI_DTYPE", "f16")
    if key not in _cache:
        _cache[key] = _build_program()
    nc = _cache[key]

    in_maps, b1 = _prep_inputs(**inputs)
    kw = {}
    if trace:
        kw["trace"] = True
        if trace_kwargs:
            kw.update(trace_kwargs)
    res = run_bass_kernel_spmd(nc, in_maps, core_ids=list(range(NCORES)), **kw)
    return _assemble(res.results, b1), res


def kernel(x, mean, cov_vector, z):
    out, _ = _run(dict(x=np.asarray(x), mean=np.asarray(mean),
                       cov_vector=np.asarray(cov_vector), z=np.asarray(z)))
    return out


# revision 7
# speedup vs baseline: 2.9229x; 2.9229x over previous
"""Trainium2 Bass kernel for nn_FCVI_Net_78864189489850.

Computation (reference):
  L = lower-tri scatter of cov_vector (exp on diag)          [769, 769]
  samples = mean + L @ z                                      [769, S, B]
  W0 = samples[0:256], b0 = samples[256:512],
  W1 = samples[512:768], b1 = samples[768]
  h = relu(x * W0 + b0);  out = sum_o h * W1 + b1             [S, B]

Strategy (8 NeuronCores, batch-sharded, no cross-device comms):
  - Host builds L, transposes to LT, casts to f16.  Each core gets a
    B-shard of z (columns c = s*256 + b_local, 4096 cols) in f16, PLUS
    256 extra rows x*z[0:256] ("z0") so the PE accumulates
    x*sT0 + sT1 straight into PSUM.
  - delta-trick: host solves LT[:, 512:768]^T delta = mean2 (min-norm)
    and ships z+delta.  The W1-side matmul then lands sT2 + mean2 in
    PSUM directly; the spurious delta terms in the W0/b0 region are
    batch-independent constants folded into apar on the host.
  - Single PSUM bank per c-tile, psAB[128, 512] = [x*sT0+sT1 | sT2+m2];
    adjacent LT column ranges stream in ONE matmul, 8 matmuls per
    c-tile, 2688 streamed PE columns (exact triangular trim).
  - Epilogue: DVE u2 = psAB[0:256] + apar; ACT h = relu(u2);
    DVE STT accumulates stag[:, m] = sum_o h * psAB[256:512].
  - The b1 row (mean768 + L[768,:] @ z) is a host-side bias added in
    _assemble; it is 0.13% of the FLOPs.
  - Output staged [128, 32], DMA'd out in 4 chunks; host reassembles
    [16, 2048] and adds b1.
"""
import os
import numpy as np

P = 769
S = 16
B = 2048
NCORES = 8
BC = B // NCORES          # 256 batch per core
NCOL = S * BC             # 4096 columns per core
NCT = NCOL // 128         # 32 c-tiles per core
ZR = 1024                 # za rows: 768 z + 256 x-scaled z

# LT columns kept per k-tile t (LT[k, i] == 0 for i < k; col 768 on host)
LT_COLS = [(0, 768), (128, 768), (256, 768), (384, 768),
           (512, 768), (640, 768)]
# z column chunks: small head so compute starts early, 1024-wide steady
CHUNKS = [(0, 128), (128, 128), (256, 256), (512, 512),
          (1024, 1024), (2048, 1024), (3072, 1024)]

_cache = {}


def _mm_dtype():
    import concourse.mybir as mybir
    name = os.environ.get("BASS_FCVI_DTYPE", "f16")
    return {
        "f16": (mybir.dt.float16, np.float16),
        "f32r": (mybir.dt.float32r, np.float32),
    }[name]


def _build_program():
    import concourse.bacc as bacc
    import concourse.tile as tile
    from concourse import mybir

    mmdt, _ = _mm_dtype()
    f32 = mybir.dt.float32

    nc = bacc.Bacc("TRN2", target_bir_lowering=False, debug=False)

    za_d = nc.dram_tensor("za", [ZR, NCOL], mmdt, kind="ExternalInput")
    lt_d = nc.dram_tensor("lt", [768, P], mmdt, kind="ExternalInput")
    cst_d = nc.dram_tensor("cst", [128, 512], f32, kind="ExternalInput")
    out_d = nc.dram_tensor("out", [128, NCT], f32, kind="ExternalOutput")

    with tile.TileContext(nc) as tc:
        with (
            tc.tile_pool(name="zpool", bufs=1) as zpool,
            tc.tile_pool(name="ltpool", bufs=1) as ltpool,
            tc.tile_pool(name="cpool", bufs=1) as cpool,
            tc.tile_pool(name="work", bufs=4) as work,
            tc.tile_pool(name="gsc", bufs=3) as gsc,
            tc.tile_pool(name="ps", bufs=6, space="PSUM") as ps_pool,
        ):
            ltt = [None] * 6
            zc = [None] * len(CHUNKS)

            def load_lt(t, eng):
                lo, hi = LT_COLS[t]
                tl = ltpool.tile([128, hi - lo], mmdt, tag=f"lt{t}")
                eng.dma_start(
                    out=tl[:], in_=lt_d.ap()[t * 128:(t + 1) * 128, lo:hi])
                ltt[t] = tl

            def load_zc(q, eng):
                cs, cn = CHUNKS[q]
                zq = zpool.tile([128, 8, cn], mmdt, tag=f"zc{q}")
                src = za_d.ap()[:, cs:cs + cn].rearrange(
                    "(t p) c -> p t c", p=128)
                eng.dma_start(out=zq[:], in_=src)
                zc[q] = zq

            # Early DMAs issued from idle engines in parallel so c-tile 0's
            # dependencies (lt0, z cols 0..128, then remaining lt) land fast.
            load_lt(0, nc.scalar)
            load_zc(0, nc.sync)
            load_lt(1, nc.gpsimd)
            load_lt(2, nc.scalar)
            load_lt(3, nc.gpsimd)
            load_lt(4, nc.scalar)
            load_lt(5, nc.gpsimd)

            cst = cpool.tile([128, 512], f32, tag="cst")
            nc.scalar.dma_start(out=cst[:], in_=cst_d.ap()[:, :])
            apar = [cst[:, 0:256], cst[:, 256:512]]

            load_zc(1, nc.sync)
            load_zc(2, nc.sync)
            for q in range(3, len(CHUNKS)):
                load_zc(q, nc.sync)

            stag = cpool.tile([128, NCT], f32, tag="stag")

            def rhs(t, g0, g1):
                lo, _ = LT_COLS[t]
                return ltt[t][:, g0 - lo:g1 - lo]

            qi = 0
            for m in range(NCT):
                par = m % 2
                if m * 128 >= CHUNKS[qi][0] + CHUNKS[qi][1]:
                    qi += 1
                cl = m * 128 - CHUNKS[qi][0]

                def lhsT(t):
                    return zc[qi][:, t, cl:cl + 128]

                # psAB[:, o]     = x*sT0[o] + sT1[o] + delta consts
                # psAB[:, 256+j] = sT2[j] + mean2[j]
                # LT cols [256, 768) of k-tiles 0..5 map linearly onto
                # psAB cols [0, 512): one stream covers both halves.
                pq = ps_pool.tile([128, 512], f32, tag="ps")
                nc.tensor.matmul(pq[:, 0:512], lhsT(0), rhs(0, 256, 768),
                                 start=True, stop=False)
                nc.tensor.matmul(pq[:, 0:256], lhsT(6), rhs(0, 0, 256),
                                 start=False, stop=False)
                nc.tensor.matmul(pq[:, 0:512], lhsT(1), rhs(1, 256, 768),
                                 start=False, stop=False)
                nc.tensor.matmul(pq[:, 128:256], lhsT(7), ltt[1][:, 0:128],
                                 start=False, stop=False)
                nc.tensor.matmul(pq[:, 0:512], lhsT(2), rhs(2, 256, 768),
                                 start=False, stop=False)
                nc.tensor.matmul(pq[:, 128:512], lhsT(3), rhs(3, 384, 768),
                                 start=False, stop=False)
                nc.tensor.matmul(pq[:, 256:512], lhsT(4), rhs(4, 512, 768),
                                 start=False, stop=False)
                nc.tensor.matmul(pq[:, 384:512], lhsT(5), rhs(5, 640, 768),
                                 start=False, stop=True)

                # u2 = x*W0 + b0 (+mean terms)   (DVE)
                u2 = work.tile([128, 256], f32, tag="u2")
                nc.vector.tensor_add(u2[:], pq[:, 0:256], apar[par])
                # h = relu(u2)   (ACT)
                h = work.tile([128, 256], f32, tag="h")
                nc.scalar.activation(h[:], u2[:],
                                     mybir.ActivationFunctionType.Relu)
                # stag[:, m] = sum_o h * (sT2 + mean2)   (DVE STT accumulate)
                gA = gsc.tile([128, 256], f32, tag="gA")
                nc.vector.scalar_tensor_tensor(
                    out=gA[:], in0=pq[:, 256:512], scalar=1.0, in1=h[:],
                    op0=mybir.AluOpType.mult, op1=mybir.AluOpType.mult,
                    accum_out=stag[:, m:m + 1])

                if m % 8 == 7:
                    sl = slice(m - 7, m + 1)
                    nc.sync.dma_start(out=out_d.ap()[:, sl], in_=stag[:, sl])

    nc.compile()
    return nc


def _prep_inputs(x, mean, cov_vector, z):
    _, npdt = _mm_dtype()

    L = np.zeros((P, P), dtype=np.float32)
    L[np.tril_indices(P)] = cov_vector
    d = np.diag(L).copy()
    L[np.diag_indices(P)] = np.exp(d)

    ltf = L.T[:768, :768].astype(np.float64)              # LT[k, i] = L[i, k]
    # delta-trick: min-norm solve of LT[:, 512:768]^T delta = mean2 so the
    # shifted z lands sT2 + mean2 in PSUM.
    m2 = mean[512:768].astype(np.float64)
    delta, *_ = np.linalg.lstsq(ltf[:, 512:768].T, m2, rcond=None)
    w = delta @ ltf                                        # [768] spurious sums
    assert np.abs(w[512:768] - m2).max() < 1e-6 * max(1.0, np.abs(m2).max())
    delta32 = delta.astype(np.float32)

    lt = np.ascontiguousarray(L.T[:768]).astype(npdt)     # rows k in [0, 768)

    z2 = z.reshape(P, S, B).astype(np.float32, copy=False)
    # b1 bias row on host: b1[s, b] = mean[768] + sum_k L[768, k] z[k, s, b]
    b1 = mean[768] + np.tensordot(L[768, :], z2, axes=1)  # [S, B]

    w = w.astype(np.float32)
    in_maps = []
    for c in range(NCORES):
        zs = z2[:, :, c * BC:(c + 1) * BC].reshape(P, NCOL)
        xs = x[c * BC:(c + 1) * BC].astype(np.float32)
        zd = zs[:768] + delta32[:, None]
        za = np.empty((ZR, NCOL), dtype=npdt)
        za[:768] = zd.astype(npdt)
        # x-prescaled z rows: column c = s*BC + b pairs with x[b]
        xcol = np.tile(xs, S)                             # [NCOL]
        za[768:1024] = (zd[:256] * xcol[None, :]).astype(npdt)
        cst = np.empty((128, 512), dtype=np.float32)
        # apar[p, o] = x_p*(mean0_o - w_o) + (mean1_o - w_{256+o}),
        # one block per batch parity
        a0 = mean[0:256] - w[0:256]
        a1 = mean[256:512] - w[256:512]
        cst[:, 0:256] = xs[0:128, None] * a0[None, :] + a1[None, :]
        cst[:, 256:512] = xs[128:256, None] * a0[None, :] + a1[None, :]
        in_maps.append({"za": za, "lt": lt, "cst": cst})
    return in_maps, b1


def _assemble(results, b1):
    out = np.empty((S, B), dtype=np.float32)
    for c in range(NCORES):
        o = results[c]["out"]                       # [128, 32]
        oc = o.reshape(128, S, 2).transpose(1, 2, 0).reshape(S, BC)
        out[:, c * BC:(c + 1) * BC] = oc
    out += b1
    return out


def _run(inputs, trace=False, trace_kwargs=None):
    from concourse.bass_utils import run_bass_kernel_spmd

    key = os.environ.get("BASS_FCVI_DTYPE", "f16")
    if key not in _cache:
        _cache[key] = _build_program()
    nc = _cache[key]

    in_maps, b1 = _prep_inputs(**inputs)
    kw = {}
    if trace:
        kw["trace"] = True
        if trace_kwargs:
            kw.update(trace_kwargs)
    res = run_bass_kernel_spmd(nc, in_maps, core_ids=list(range(NCORES)), **kw)
    return _assemble(res.results, b1), res


def kernel(x, mean, cov_vector, z):
    out, _ = _run(dict(x=np.asarray(x), mean=np.asarray(mean),
                       cov_vector=np.asarray(cov_vector), z=np.asarray(z)))
    return out
